# revision 8
# baseline (speedup 1.0000x reference)
"""Deformable temporal attention on 8 trn2 NeuronCores.

Sharding: core c handles batch b = c // 2 and head-group g = c % 2
(heads 4g..4g+3). Each core returns a partial (4096, 256) output
(its 4 heads' contribution through W_o, with b_o folded into g==0);
host sums the two partials per batch.

Math note: the reference's sampling grid and attention weights do not
depend on the frame t, and bilinear sampling is linear in the image, so
sum_t bilinear(value_t) = bilinear(sum_t value_t) and
sum_t value_t = (sum_t x_t) @ W_v + T*b_v.
"""
import sys
sys.path.insert(0, '/opt/trn_rl_repo')

import numpy as np
from contextlib import ExitStack

import concourse.bass as bass
import concourse.bacc as bacc
import concourse.tile as tile
import concourse.mybir as mybir
from concourse import bass_utils
from concourse._compat import with_exitstack

F32 = mybir.dt.float32
F32R = mybir.dt.float32r
BF16 = mybir.dt.float16  # 16-bit value/weight pipeline dtype
I32 = mybir.dt.int32
I16 = mybir.dt.int16
OP = mybir.AluOpType
AF = mybir.ActivationFunctionType
AX = mybir.AxisListType

B, N, T, D = 4, 4096, 3, 256
HH, PP = 8, 9            # total heads, points
HPC = 4                  # heads per core
HP = WP = 64             # spatial grid
NT = N // 128            # 32 n-tiles
K = NT * PP              # 288 samples per partition per head
MAGIC = 8388608.0        # 2^23: (x + MAGIC) - MAGIC == round(x) for |x| << 2^23
RMAX = 62 * 64 + 62      # max gather row index after clamping


def _mkap(base: bass.AP, ap_list):
    return bass.AP(base.tensor, base.offset, ap_list)


def _load_consts(nc, pool, io):
    t = {}
    specs = [("wcat", [128, 2, 108], "r2"), ("wv", [128, 2, 128], "r2"),
             ("wo", [128, 256], ""), ("bcat", [128, 108], ""),
             ("bv", [128, 128], ""), ("bo", [128, 256], ""),
             ("refx", [128, 1], ""), ("refy0", [128, 1], ""),
             ("ntramp", [128, 32], ""), ("ident", [128, 128], "")]
    for nm, shape, kind in specs:
        tl = pool.tile(shape, F32, tag=nm, name=nm + "_sb")
        src = io[nm].ap()
        if kind == "r2":
            src = src.rearrange("(c k) m -> k c m", k=128)
        nc.sync.dma_start(tl[:], src)
        if nm in ("wcat", "wv", "wo"):
            tr = pool.tile(shape, F32R, tag=nm + "r", name=nm + "_r")
            nc.vector.tensor_copy(tr[:], tl[:])
            t[nm] = tr
        else:
            t[nm] = tl
    return t


def _weight_pipe(nc, wp, off_all, cs, h):
    """Per-head weight pipeline. Returns (idx_t, w4b)."""
    offx = off_all[:, :, h * PP:(h + 1) * PP]
    offy = off_all[:, :, 36 + h * PP:36 + (h + 1) * PP]
    lgts = off_all[:, :, 72 + h * PP:72 + (h + 1) * PP]
    sh9 = [128, NT, PP]

    gx = wp.tile(sh9, F32, tag="gx", name="gx")
    nc.vector.tensor_scalar(gx[:], offx, 31.5, cs["refx"][:],
                            op0=OP.mult, op1=OP.add)
    gy = wp.tile(sh9, F32, tag="gy", name="gy")
    nc.vector.tensor_scalar(gy[:], offy, 31.5, cs["refy0"][:],
                            op0=OP.mult, op1=OP.add)
    ntb = _mkap(cs["ntramp"][:], cs["ntramp"][:].ap + [[0, PP]])
    nc.vector.tensor_tensor(out=gy[:], in0=gy[:], in1=ntb, op=OP.add)

    # x0 = clamp(floor(gx), 0, 62); floor = round(gx) - (round(gx) > gx)
    def floor_clamp(g, tagp):
        r = wp.tile(sh9, F32, tag=tagp + "r", name=tagp + "r")
        nc.vector.tensor_scalar(r[:], g[:], MAGIC, MAGIC,
                                op0=OP.add, op1=OP.subtract)
        m = wp.tile(sh9, F32, tag=tagp + "m", name=tagp + "m")
        nc.vector.tensor_tensor(out=m[:], in0=r[:], in1=g[:], op=OP.is_gt)
        nc.vector.tensor_tensor(out=r[:], in0=r[:], in1=m[:], op=OP.subtract)
        nc.vector.tensor_scalar(r[:], r[:], 0.0, 62.0, op0=OP.max, op1=OP.min)
        return r
    x0 = floor_clamp(gx, "x0")
    y0 = floor_clamp(gy, "y0")

    idxf = wp.tile(sh9, F32, tag="idxf", name="idxf")
    nc.vector.tensor_scalar(idxf[:], y0[:], 64.0, None, op0=OP.mult)
    nc.vector.tensor_tensor(out=idxf[:], in0=idxf[:], in1=x0[:], op=OP.add)
    # int16 indices, then rewrap to dma_gather's (16, num/16) layout
    # (sample s lives at [s % 16, s // 16]; s = k*128 + q so that the
    # gathered row for (q, k) lands on partition q, block k), finally
    # replicate across the 8 Q7 core partition groups.
    idx16 = wp.tile([128, K], I16, tag="idx16", name="idx16")
    nc.vector.tensor_copy(idx16[:], idxf[:].rearrange("p a b -> p (a b)"))
    tmpw = wp.tile([16, 8, K], I16, tag="tmpw", name="tmpw")
    for qhi in range(8):
        nc.sync.dma_start(tmpw[0:16, qhi, :], idx16[16 * qhi:16 * qhi + 16, :])
    gidx = wp.tile([128, 8 * K], I16, tag="gidx", name="gidx")
    tsrc = _mkap(tmpw[:], [tmpw[:].ap[0], [1, K], [K, 8]])
    nc.vector.tensor_copy(gidx[0:16, :], tsrc)
    for rep in range(1, 8):
        nc.sync.dma_start(gidx[16 * rep:16 * rep + 16, :], gidx[0:16, :])

    # hat weights: w0 = relu(1 - |d|) = relu(min(1-d, 1+d)),
    #              w1 = relu(1 - |d-1|) = relu(min(2-d, d)),  d = g - z0
    def hats(g, z0, tagp):
        d = wp.tile(sh9, F32, tag=tagp + "d", name=tagp + "d")
        nc.vector.tensor_tensor(out=d[:], in0=g[:], in1=z0[:], op=OP.subtract)
        a0 = wp.tile(sh9, F32, tag=tagp + "a0", name=tagp + "a0")
        nc.vector.tensor_scalar(a0[:], d[:], -1.0, 1.0,
                                op0=OP.mult, op1=OP.add)
        b0 = wp.tile(sh9, F32, tag=tagp + "b0", name=tagp + "b0")
        nc.vector.tensor_scalar(b0[:], d[:], 1.0, None, op0=OP.add)
        nc.vector.tensor_tensor(out=a0[:], in0=a0[:], in1=b0[:], op=OP.min)
        w0 = wp.tile(sh9, F32, tag=tagp + "w0", name=tagp + "w0")
        nc.scalar.activation(w0[:], a0[:], AF.Relu)
        a1 = wp.tile(sh9, F32, tag=tagp + "a1", name=tagp + "a1")
        nc.vector.tensor_scalar(a1[:], d[:], -1.0, 2.0,
                                op0=OP.mult, op1=OP.add)
        nc.vector.tensor_tensor(out=a1[:], in0=a1[:], in1=d[:], op=OP.min)
        w1 = wp.tile(sh9, F32, tag=tagp + "w1", name=tagp + "w1")
        nc.scalar.activation(w1[:], a1[:], AF.Relu)
        return w0, w1
    wx0, wx1 = hats(gx, x0, "hx")
    wy0, wy1 = hats(gy, y0, "hy")

    # softmax over the 9 points
    mx = wp.tile([128, NT], F32, tag="mx", name="mx")
    nc.vector.reduce_max(mx[:], lgts, axis=AX.X)
    el = wp.tile(sh9, F32, tag="el", name="el")
    mxb = _mkap(mx[:], mx[:].ap + [[0, PP]])
    nc.vector.tensor_tensor(out=el[:], in0=lgts, in1=mxb, op=OP.subtract)
    nc.scalar.activation(el[:], el[:], AF.Exp)
    sm = wp.tile([128, NT], F32, tag="sm", name="sm")
    nc.vector.reduce_sum(sm[:], el[:], axis=AX.X)
    nc.vector.reciprocal(sm[:], sm[:])
    smb = _mkap(sm[:], sm[:].ap + [[0, PP]])
    attn = wp.tile(sh9, F32, tag="attn", name="attn")
    nc.vector.tensor_tensor(out=attn[:], in0=el[:], in1=smb, op=OP.mult)

    # corner weights, corner order [x0y0, x1y0, x0y1, x1y1]
    nc.vector.tensor_tensor(out=wy0[:], in0=wy0[:], in1=attn[:], op=OP.mult)
    nc.vector.tensor_tensor(out=wy1[:], in0=wy1[:], in1=attn[:], op=OP.mult)
    w4f = wp.tile([128, K, 4], F32, tag="w4f", name="w4f")
    w4v = w4f[:].rearrange("p (a b) c -> p a b c", a=NT)  # (128, NT, PP, 4)
    for ci, (wya, wxa) in enumerate(((wy0, wx0), (wy0, wx1),
                                     (wy1, wx0), (wy1, wx1))):
        nc.vector.tensor_tensor(out=w4v[:, :, :, ci], in0=wya[:],
                                in1=wxa[:], op=OP.mult)
    w4b = wp.tile([128, K, 4], BF16, tag="w4b", name="w4b")
    nc.vector.tensor_copy(w4b[:], w4f[:])
    return gidx, w4b


@with_exitstack
def _kernel_body(ctx: ExitStack, tc: tile.TileContext, io: dict):
    nc = tc.nc
    xb = io["xb"].ap()
    out = io["out_part"].ap()
    v_dram = [io[f"v{h}"].ap() for h in range(HPC)]
    v4_dram = [io[f"v4_{h}"].ap() for h in range(HPC)]

    consts = ctx.enter_context(tc.tile_pool(name="consts", bufs=1))
    cs = _load_consts(nc, consts, io)

    offall = ctx.enter_context(tc.tile_pool(name="offall", bufs=1))
    off_all = offall.tile([128, NT, 108], F32, tag="offa", name="off_all")
    vb_all = offall.tile([128, NT, 128], BF16, tag="vball", name="vb_all")
    sall = ctx.enter_context(tc.tile_pool(name="sall", bufs=1))
    s_all = sall.tile([128, NT, 128], F32, tag="sall", name="s_all")

    # ---- Phases A+B: load, transpose, project ----
    xg = xb.rearrange("(nt p) (t d) -> p nt t d", p=128, t=T)
    with tc.tile_pool(name="tmat", bufs=1) as tmat:
        qT = [tmat.tile([128, N], F32R, tag=f"qT{c}", name=f"qT{c}")
              for c in range(2)]
        xsT = [tmat.tile([128, N], F32R, tag=f"xsT{c}", name=f"xsT{c}")
               for c in range(2)]
        with tc.tile_pool(name="xload", bufs=1) as xload:
            xs_all = xload.tile([128, NT, 256], F32, tag="xs", name="xs_all")
            q_all = xload.tile([128, NT, 256], F32, tag="q", name="q_all")
            nc.gpsimd.dma_start(xs_all[:], xg[:, :, 0, :])
            nc.gpsimd.dma_start(xs_all[:], xg[:, :, 1, :], accum_op=OP.add)
            nc.gpsimd.dma_start(xs_all[:], xg[:, :, 2, :], accum_op=OP.add)
            nc.sync.dma_start(q_all[:], xg[:, :, 1, :])

            with tc.tile_pool(name="tps", bufs=4, space="PSUM") as tps:
                for (src, dstl) in ((q_all, qT), (xs_all, xsT)):
                    for c in range(2):
                        for g4 in range(NT // 4):
                            pt = tps.tile([128, 512], F32, tag="pt",
                                          name="pt")
                            for j in range(4):
                                nt = g4 * 4 + j
                                nc.tensor.transpose(
                                    out=pt[:, j * 128:(j + 1) * 128],
                                    in_=src[:, nt, c * 128:(c + 1) * 128],
                                    identity=cs["ident"][:])
                            nc.any.tensor_copy(
                                dstl[c][:, g4 * 512:(g4 + 1) * 512], pt[:])

        with tc.tile_pool(name="pps", bufs=4, space="PSUM") as pps:
            for nt in range(NT):
                poa = pps.tile([128, 108], F32, tag="poa", name="poa")
                for c in range(2):
                    nc.tensor.matmul(
                        poa[:],
                        lhsT=qT[c][:, nt * 128:(nt + 1) * 128],
                        rhs=cs["wcat"][:, c, :],
                        start=(c == 0), stop=(c == 1))
                nc.vector.tensor_tensor(
                    out=off_all[:, nt, :], in0=poa[:], in1=cs["bcat"][:],
                    op=OP.add)
                pv = pps.tile([128, 128], F32, tag="pv", name="pv")
                for c in range(2):
                    nc.tensor.matmul(
                        pv[:],
                        lhsT=xsT[c][:, nt * 128:(nt + 1) * 128],
                        rhs=cs["wv"][:, c, :],
                        start=(c == 0), stop=(c == 1))
                nc.vector.tensor_tensor(
                    out=vb_all[:, nt, :], in0=pv[:], in1=cs["bv"][:],
                    op=OP.add)

    # ---- value images to DRAM; V4 quad expansion ----
    for h in range(HPC):
        dst = v_dram[h].rearrange("(nt p) c -> p nt c", p=128)
        nc.sync.dma_start(dst, vb_all[:, :, h * 32:(h + 1) * 32])
        src = _mkap(v_dram[h], [[32, RMAX + 1], [64 * 32, 2], [32, 2],
                                [1, 32]])
        dst4 = _mkap(v4_dram[h], [[128, RMAX + 1], [1, 128]])
        nc.sync.dma_start(dst4, src)

    # ---- Phases C/D per head: weights, gather, weighted reduce ----
    with tc.tile_pool(name="wpipe", bufs=1) as wp, \
         tc.tile_pool(name="gpool", bufs=2) as gp:
        for h in range(HPC):
            gidx, w4b = _weight_pipe(nc, wp, off_all, cs, h)
            # dma_gather crashes above ~14336 idxs (32KB/partition out);
            # use nt-aligned chunks of <= 12 n-tiles (13824 samples).
            for nt0, nt1 in ((0, 12), (12, 24), (24, NT)):
                nnt = nt1 - nt0
                kh = nnt * PP
                ks = slice(nt0 * PP, nt1 * PP)
                g = gp.tile([128, 12 * PP, 128], BF16, tag="G", name="G")
                gs = g[:, 0:kh, :]
                ni = kh * 128
                nc.gpsimd.dma_gather(
                    out_ap=gs, in_ap=v4_dram[h],
                    idxs_ap=gidx[:, nt0 * PP * 8:nt1 * PP * 8],
                    num_idxs=ni, num_idxs_reg=ni, elem_size=128,
                    single_packet=False)
                w4s = w4b[:, ks, :]
                w4x = _mkap(w4s, w4s.ap + [[0, 32]])
                gv = gs.rearrange("p k (a c) -> p k a c", a=4)
                nc.vector.tensor_tensor(out=gv[:], in0=gv[:], in1=w4x,
                                        op=OP.mult)
                nc.vector.tensor_tensor(
                    out=gs[:, :, 0:64], in0=gs[:, :, 0:64],
                    in1=gs[:, :, 64:128], op=OP.add)
                nc.vector.tensor_tensor(
                    out=gs[:, :, 0:32], in0=gs[:, :, 0:32],
                    in1=gs[:, :, 32:64], op=OP.add)
                pv4 = gs.rearrange("p (a b) c -> p a b c", b=PP)
                nc.vector.tensor_tensor(
                    out=pv4[:, :, 0:4, 0:32], in0=pv4[:, :, 0:4, 0:32],
                    in1=pv4[:, :, 4:8, 0:32], op=OP.add)
                nc.vector.tensor_tensor(
                    out=pv4[:, :, 0:2, 0:32], in0=pv4[:, :, 0:2, 0:32],
                    in1=pv4[:, :, 2:4, 0:32], op=OP.add)
                nc.vector.tensor_tensor(
                    out=pv4[:, :, 0:1, 0:32], in0=pv4[:, :, 0:1, 0:32],
                    in1=pv4[:, :, 1:2, 0:32], op=OP.add)
                nc.vector.tensor_tensor(
                    out=s_all[:, nt0:nt1, h * 32:(h + 1) * 32],
                    in0=pv4[:, :, 0, 0:32], in1=pv4[:, :, 8, 0:32],
                    op=OP.add)

    # ---- Phase E: out projection ----
    with tc.tile_pool(name="stp", bufs=1) as stp:
        st = stp.tile([128, N], F32R, tag="st", name="st")
        with tc.tile_pool(name="eps", bufs=4, space="PSUM") as eps:
            for g4 in range(NT // 4):
                pt = eps.tile([128, 512], F32, tag="ept", name="ept")
                for j in range(4):
                    nt = g4 * 4 + j
                    nc.tensor.transpose(
                        out=pt[:, j * 128:(j + 1) * 128],
                        in_=s_all[:, nt, :], identity=cs["ident"][:])
                nc.any.tensor_copy(st[:, g4 * 512:(g4 + 1) * 512], pt[:])
            with tc.tile_pool(name="otp", bufs=3) as otp:
                for nt in range(NT):
                    po = eps.tile([128, 256], F32, tag="epo", name="epo")
                    nc.tensor.matmul(
                        po[:],
                        lhsT=st[:, nt * 128:(nt + 1) * 128],
                        rhs=cs["wo"][:], start=True, stop=True)
                    ot = otp.tile([128, 256], F32, tag="ot", name="ot")
                    nc.vector.tensor_tensor(out=ot[:], in0=po[:],
                                            in1=cs["bo"][:], op=OP.add)
                    nc.sync.dma_start(out[nt * 128:(nt + 1) * 128, :], ot[:])


def build_program():
    nc = bacc.Bacc("TRN2", target_bir_lowering=False, debug=False,
                   num_devices=8)
    io = {}
    io["xb"] = nc.dram_tensor("xb", [N, T * D], F32, kind="ExternalInput")
    io["wcat"] = nc.dram_tensor("wcat", [D, 108], F32, kind="ExternalInput")
    io["wv"] = nc.dram_tensor("wv", [D, 128], F32, kind="ExternalInput")
    io["wo"] = nc.dram_tensor("wo", [128, 256], F32, kind="ExternalInput")
    io["bcat"] = nc.dram_tensor("bcat", [128, 108], F32, kind="ExternalInput")
    io["bv"] = nc.dram_tensor("bv", [128, 128], F32, kind="ExternalInput")
    io["bo"] = nc.dram_tensor("bo", [128, 256], F32, kind="ExternalInput")
    io["refx"] = nc.dram_tensor("refx", [128, 1], F32, kind="ExternalInput")
    io["refy0"] = nc.dram_tensor("refy0", [128, 1], F32, kind="ExternalInput")
    io["ntramp"] = nc.dram_tensor("ntramp", [128, 32], F32,
                                  kind="ExternalInput")
    io["ident"] = nc.dram_tensor("ident", [128, 128], F32,
                                 kind="ExternalInput")
    for h in range(HPC):
        io[f"v{h}"] = nc.dram_tensor(f"v{h}", [N, 32], BF16)
        io[f"v4_{h}"] = nc.dram_tensor(f"v4_{h}", [N, 128], BF16)
    io["out_part"] = nc.dram_tensor("out_part", [N, 256], F32,
                                    kind="ExternalOutput")
    with tile.TileContext(nc) as tc:
        _kernel_body(tc, io)
    nc.compile()
    return nc


def make_in_maps(x, W_off, b_off, W_attn, b_attn, W_v, b_v, W_o, b_o):
    """Build the 8 per-core input maps from full inputs."""
    p = np.arange(128, dtype=np.float32)
    refx = (p % 64).reshape(128, 1)
    refy0 = (p // 64).reshape(128, 1)
    ntramp = np.broadcast_to((2.0 * np.arange(NT, dtype=np.float32)),
                             (128, NT)).copy()
    ident = np.eye(128, dtype=np.float32)

    woff_r = W_off.reshape(D, HH, PP, 2)
    wattn_r = W_attn.reshape(D, HH, PP)
    boff_r = b_off.reshape(HH, PP, 2)
    battn_r = b_attn.reshape(HH, PP)
    wv_r = W_v.reshape(D, HH, 32)
    bv_r = b_v.reshape(HH, 32)
    wo_r = W_o.reshape(HH, 32, 256)

    in_maps = []
    for c in range(8):
        b, g = c // 2, c % 2
        hs = slice(g * HPC, (g + 1) * HPC)
        wcat = np.concatenate([
            woff_r[:, hs, :, 0].reshape(D, 36),
            woff_r[:, hs, :, 1].reshape(D, 36),
            wattn_r[:, hs, :].reshape(D, 36)], axis=1)
        bcat = np.concatenate([
            boff_r[hs, :, 0].reshape(36),
            boff_r[hs, :, 1].reshape(36),
            battn_r[hs, :].reshape(36)])
        wv = wv_r[:, hs, :].reshape(D, 128)
        bv = (float(T) * bv_r[hs, :]).reshape(128)
        wo = wo_r[hs].reshape(128, 256)
        bo = b_o if g == 0 else np.zeros_like(b_o)
        in_maps.append({
            "xb": np.ascontiguousarray(x[b].reshape(N, T * D)),
            "wcat": np.ascontiguousarray(wcat),
            "wv": np.ascontiguousarray(wv),
            "wo": np.ascontiguousarray(wo),
            "bcat": np.broadcast_to(bcat, (128, 108)).copy(),
            "bv": np.broadcast_to(bv, (128, 128)).copy(),
            "bo": np.broadcast_to(bo, (128, 256)).copy(),
            "refx": refx, "refy0": refy0, "ntramp": ntramp, "ident": ident,
        })
    return in_maps


_NC_CACHE = None


def kernel(x, W_off, b_off, W_attn, b_attn, W_v, b_v, W_o, b_o, Hp, Wp):
    global _NC_CACHE
    assert int(Hp) == HP and int(Wp) == WP
    x = np.asarray(x, dtype=np.float32)
    args = [np.asarray(a, dtype=np.float32)
            for a in (W_off, b_off, W_attn, b_attn, W_v, b_v, W_o, b_o)]
    if _NC_CACHE is None:
        _NC_CACHE = build_program()
    in_maps = make_in_maps(x, *args)
    res = bass_utils.run_bass_kernel_spmd(
        _NC_CACHE, in_maps, core_ids=list(range(8)))
    out = np.empty((B, N, D), dtype=np.float32)
    for b in range(B):
        out[b] = (res.results[2 * b]["out_part"]
                  + res.results[2 * b + 1]["out_part"])
    return out
